# revision 18
# baseline (speedup 1.0000x reference)
"""DOM pooling (segment mean+max over pulses, then linear projection) on 8 trn2 cores.

Strategy (v2, fp16 feature-major):
  Host: bucket DOMs by exact pulse count k ("classes"); deal DOMs of each
  class round-robin across the 8 cores so every core has identical structure
  (per-class m = ceil(n_k/8) doms, zero-padded). Windows of 128 doms per
  class, organized as two halves of <=64 doms. Slot buffers are fp16,
  feature-major: partition p = half*64 + embed, free = (window, slot, dom).
  Partial windows use D_w = ceil(rem/2) dom columns per half. No ragged
  sharing, no pad corrections (padding is all-zeros -> dummy doms only).

  Device (one NEFF, SPMD on 8 cores), per class window-group:
    - one contiguous fp16 DMA load per group (up to 32KB/partition runs)
    - segment SUM fused into the projection: k accumulating PE matmuls with
      block-diagonal fp16 weights blkdiag(W_sum/k) -> PSUM holds the
      projected mean contribution (scaling folded into weights)
    - segment MAX as a pairwise fp16 tensor_tensor tree on DVE (2x mode),
      then one more accumulating matmul with blkdiag(W_max) closes PSUM
    - ACT adds bias during PSUM->SBUF copy (fp16 out); per-group DMA store
  Partial windows compute both sum and max trees on DVE (tiny) + 2 matmuls.

  Host: scatter per-core outputs [128=(half,e), cols] back to (num_doms, 64).
"""
import sys

import numpy as np

for _p in ("/opt/trn_rl_repo",):
    if _p not in sys.path:
        sys.path.append(_p)

from concourse import bacc
import concourse.mybir as mybir
import concourse.tile as tile
from concourse.bass_utils import run_bass_kernel_spmd

NCORES = 8
D = 64
FP32 = mybir.dt.float32
FP16 = mybir.dt.float16

last_exec_ns = None  # set when KERNEL_TRACE=1


def _plan(counts):
    """Shared class/window structure (derived from global counts).

    Returns list of class dicts and totals. Per class k:
      m: doms per core (ceil), fw: full 128-dom windows, rem: leftover doms,
      dw: per-half columns of the partial window (ceil(rem/2)),
      base_f/base_p: slot-buffer element offsets (full / partial region),
      hcol0: first output column (per-half column space).
    """
    kmax = int(counts.max()) if counts.size else 0
    n_k = np.bincount(counts, minlength=kmax + 1)
    classes = []
    base = 0
    hcol = 0
    for k in range(1, kmax + 1):
        if n_k[k] == 0:
            continue
        m = -(-int(n_k[k]) // NCORES)
        fw = m // 128
        rem = m % 128
        dw = -(-rem // 2)
        c = dict(k=k, n=int(n_k[k]), m=m, fw=fw, rem=rem, dw=dw,
                 base_f=base, hcol0=hcol)
        base += 128 * fw * k * D
        hcol += fw * D
        c["base_p"] = base
        if rem:
            base += 128 * k * dw
            hcol += dw
        classes.append(c)
    return classes, base, hcol


def _emit_tree(eng, nc, v, k, gwa, dcols, out4, op, workp, tagp):
    """Pairwise reduction tree over the slot axis.

    v: 4D view (p, w=gwa, s=k, d=dcols); out4: (p, w, 1, d) destination view.
    Emits ceil(log2 k) fp16 tensor_tensor levels (+copies for odd carries).
    Caller must handle k == 1 (no op needed).
    """
    assert k >= 2
    cur = v
    s = k
    lvl = 0
    while s > 1:
        b = s // 2
        odd = s & 1
        tgt = b + odd
        if b == 1 and odd == 0:
            eng.tensor_tensor(out=out4[:, :, 0:1, :], in0=cur[:, :, 0:1, :],
                              in1=cur[:, :, 1:2, :], op=op)
            return
        wt = workp.tile([128, gwa * tgt * dcols], FP16, tag=f"{tagp}{lvl % 2}")
        dst = wt[:].rearrange("p (w s d) -> p w s d", w=gwa, s=tgt)
        eng.tensor_tensor(out=dst[:, :, 0:b, :], in0=cur[:, :, 0:b, :],
                          in1=cur[:, :, b : 2 * b, :], op=op)
        if odd:
            eng.tensor_copy(dst[:, :, b : b + 1, :], cur[:, :, 2 * b : 2 * b + 1, :])
        cur = dst
        s = tgt
        lvl += 1


def _build_nc(classes, s_elems, ncolh):
    ncls = len(classes)
    nc = bacc.Bacc(None)
    slots_t = nc.dram_tensor("slots", [s_elems], FP16, kind="ExternalInput")
    wts_t = nc.dram_tensor("wts", [(ncls + 1) * 128, 128], FP16, kind="ExternalInput")
    b_t = nc.dram_tensor("b", [128, 1], FP32, kind="ExternalInput")
    out_t = nc.dram_tensor("out", [128, ncolh], FP16, kind="ExternalOutput")

    with tile.TileContext(nc) as tc:
        with (
            tc.tile_pool(name="const", bufs=1) as constp,
            tc.tile_pool(name="inp", bufs=4) as inp,
            tc.tile_pool(name="pin", bufs=4) as pinp,
            tc.tile_pool(name="work", bufs=1) as workp,
            tc.tile_pool(name="redg", bufs=2) as redp,
            tc.tile_pool(name="outp", bufs=2) as outp,
            tc.tile_pool(name="ps", bufs=4, space="PSUM") as psp,
        ):
            wk_sb = constp.tile([128, (ncls + 1) * 128], FP16)
            nc.scalar.dma_start(
                wk_sb[:].rearrange("p (j m) -> p j m", m=128),
                wts_t[:, :].rearrange("(j p) m -> p j m", p=128),
            )
            b_sb = constp.tile([128, 1], FP32)
            nc.scalar.dma_start(b_sb[:], b_t[:])

            def w_ap(j):
                return wk_sb[:, j * 128 : (j + 1) * 128]

            wmax_j = ncls  # last weight block = blkdiag(W_max), unscaled

            def do_windows(jcls, k, in_t, gwa, dcols, base_col):
                """Reduce+project gwa windows of dcols half-columns each."""
                v = in_t[:].rearrange("p (w s d) -> p w s d", w=gwa, s=k)
                use_tree = k > 1
                if use_tree:
                    maxg = redp.tile([128, gwa * dcols], FP16, tag="maxg")
                    mg4 = maxg[:].rearrange("p (w s d) -> p w s d", w=gwa, s=1)
                    mg3 = mg4[:, :, 0, :]
                    _emit_tree(nc.vector, nc, v, k, gwa, dcols,
                               mg4, mybir.AluOpType.max, workp, "m")
                out_sb = outp.tile([128, gwa * dcols], FP16, tag="out")
                for w0 in range(0, gwa, 8):
                    sgw = min(8, gwa - w0)
                    N = sgw * dcols
                    ps = psp.tile([128, N], FP32, space="PSUM", tag="ps")
                    for s in range(k):
                        nc.tensor.matmul(
                            ps[:, :N], lhsT=w_ap(jcls),
                            rhs=v[:, w0 : w0 + sgw, s, :],
                            start=(s == 0), stop=False,
                        )
                    if use_tree:
                        rhs_max = mg3[:, w0 : w0 + sgw, :]
                    else:
                        rhs_max = v[:, w0 : w0 + sgw, 0, :]
                    nc.tensor.matmul(
                        ps[:, :N], lhsT=w_ap(wmax_j), rhs=rhs_max,
                        start=False, stop=True,
                    )
                    nc.scalar.activation(
                        out_sb[:, w0 * dcols : w0 * dcols + N], ps[:, :N],
                        mybir.ActivationFunctionType.Identity, bias=b_sb[:, :1],
                    )
                nc.gpsimd.dma_start(
                    out_t[:, base_col : base_col + gwa * dcols],
                    out_sb[:, : gwa * dcols],
                )

            def emit_partial(jcls):
                c = classes[jcls]
                k, fw, dw = c["k"], c["fw"], c["dw"]
                F0 = k * dw
                in_t = pinp.tile([128, F0], FP16, tag="pin")
                nc.sync.dma_start(
                    in_t[:],
                    slots_t[c["base_p"] : c["base_p"] + 128 * F0]
                    .rearrange("(p f) -> p f", p=128),
                )
                do_windows(jcls, k, in_t, 1, dw, c["hcol0"] + fw * D)

            # full-window groups (largest classes first), with the small
            # partial windows interleaved so their latency-bound chains hide
            # under the DMA-dense phase
            order = sorted(range(len(classes)),
                           key=lambda jj: -classes[jj]["fw"] * classes[jj]["k"])
            partials = [jj for jj, c in enumerate(classes) if c["rem"]]
            # keep the smallest few partials for the very end: their tiny
            # loads keep DMA trickling while the full-group backlog drains
            ntail = min(6, len(partials))
            head_p = partials[: len(partials) - ntail]
            tail_p = partials[len(partials) - ntail :]
            pi = 0
            for jcls in order:
                c = classes[jcls]
                k, fw = c["k"], c["fw"]
                if not fw:
                    continue
                gw = max(1, 192 // k)
                F_cls = fw * k * D
                full2d = slots_t[c["base_f"] : c["base_f"] + 128 * F_cls] \
                    .rearrange("(p f) -> p f", p=128)
                for g0 in range(0, fw, gw):
                    gwa = min(gw, fw - g0)
                    F0 = gwa * k * D
                    in_t = inp.tile([128, F0], FP16, tag="in")
                    nc.sync.dma_start(
                        in_t[:], full2d[:, g0 * k * D : g0 * k * D + F0]
                    )
                    do_windows(jcls, k, in_t, gwa, D, c["hcol0"] + g0 * D)
                    if pi < len(head_p):
                        emit_partial(head_p[pi])
                        pi += 1
            while pi < len(head_p):
                emit_partial(head_p[pi])
                pi += 1
            for jj in tail_p:
                emit_partial(jj)
    nc.finalize()
    return nc


def kernel(pulse_embeddings, pulse_to_dom_idx, num_doms, proj_w, proj_b):
    global last_exec_ns
    import os

    E = np.asarray(pulse_embeddings, dtype=np.float32)
    E16 = E.astype(np.float16)
    idx = np.asarray(pulse_to_dom_idx).astype(np.int64)
    nd = int(num_doms)
    W = np.asarray(proj_w, dtype=np.float32)   # (D, 2D)
    b = np.asarray(proj_b, dtype=np.float32)   # (D,)

    counts = np.bincount(idx, minlength=nd)
    classes, s_elems, ncolh = _plan(counts)
    ncls = len(classes)

    dom_order = np.argsort(counts, kind="stable")
    n0 = int((counts == 0).sum())
    perm = np.argsort(idx, kind="stable")
    pstart = np.zeros(nd + 1, np.int64)
    pstart[1:] = np.cumsum(counts)

    # per-dom output routing (core, half, halfcol) for real doms
    dom_core = np.full(nd, -1, np.int32)
    dom_half = np.zeros(nd, np.int32)
    dom_hcol = np.zeros(nd, np.int32)

    bufs = [np.zeros(s_elems, np.float16) for _ in range(NCORES)]
    off = n0
    for c in classes:
        k, n, m, fw, rem, dw = c["k"], c["n"], c["m"], c["fw"], c["rem"], c["dw"]
        doms_k = dom_order[off : off + n]
        off += n
        # routing: class-list index i -> core i%8, position p=i//8
        i_arr = np.arange(n, dtype=np.int64)
        p_arr = i_arr // NCORES
        dom_core[doms_k] = (i_arr % NCORES).astype(np.int32)
        isfull = p_arr < fw * 128
        q = np.where(isfull, p_arr % 128, p_arr - fw * 128)
        halfsz = np.where(isfull, 64, dw)
        dom_half[doms_k] = (q // halfsz).astype(np.int32)
        dcol = q % halfsz
        dom_hcol[doms_k] = np.where(
            isfull, c["hcol0"] + (p_arr // 128) * D + dcol,
            c["hcol0"] + fw * D + dcol,
        ).astype(np.int32)

        for cc in range(NCORES):
            doms_c = doms_k[cc::NCORES]
            nreal = len(doms_c)
            rows = pstart[doms_c][:, None] + np.arange(k)[None, :]
            X = E16[perm[rows]]  # (nreal, k, 64)
            if nreal < m:
                X = np.concatenate(
                    [X, np.zeros((m - nreal, k, D), np.float16)], axis=0
                )
            if fw:
                Xf = X[: fw * 128].reshape(fw, 2, 64, k, D)  # w h d s e
                arr = Xf.transpose(1, 4, 0, 3, 2)            # h e w s d
                bufs[cc][c["base_f"] : c["base_f"] + 128 * fw * k * D] = \
                    np.ascontiguousarray(arr).reshape(-1)
            if rem:
                Xr = X[fw * 128 :]  # (rem, k, D)
                if rem < 2 * dw:
                    Xr = np.concatenate(
                        [Xr, np.zeros((2 * dw - rem, k, D), np.float16)], axis=0
                    )
                arr = Xr.reshape(2, dw, k, D).transpose(0, 3, 2, 1)  # h e s d
                bufs[cc][c["base_p"] : c["base_p"] + 128 * k * dw] = \
                    np.ascontiguousarray(arr).reshape(-1)

    # ---- weights: per-class blkdiag(W_sum/k), plus blkdiag(W_max) --------
    Wsum = W[:, :D]   # (out_e, feat_e)
    Wmax = W[:, D:]
    wts = np.zeros(((ncls + 1) * 128, 128), np.float16)
    for j, c in enumerate(classes):
        blk = (Wsum.T / np.float32(c["k"])).astype(np.float16)  # (feat, out)
        wts[j * 128 : j * 128 + 64, 0:64] = blk
        wts[j * 128 + 64 : (j + 1) * 128, 64:128] = blk
    blk = Wmax.T.astype(np.float16)
    wts[ncls * 128 : ncls * 128 + 64, 0:64] = blk
    wts[ncls * 128 + 64 :, 64:128] = blk
    b_col = np.tile(b, 2).reshape(128, 1).astype(np.float32)

    # ---- device ----------------------------------------------------------
    nc = _build_nc(classes, s_elems, ncolh)
    in_maps = [{"slots": bufs[cc], "wts": wts, "b": b_col} for cc in range(NCORES)]
    trace = os.environ.get("KERNEL_TRACE", "0") == "1"
    kw_ = {}
    if trace:
        import tempfile
        kw_ = dict(trace=True, tmpdir=tempfile.mkdtemp(prefix="kernel_trace_"))
    res = run_bass_kernel_spmd(nc, in_maps, core_ids=list(range(NCORES)), **kw_)
    last_exec_ns = res.exec_time_ns

    # ---- host-side unpermute --------------------------------------------
    outs = np.stack([res.results[cc]["out"] for cc in range(NCORES)]) \
        .astype(np.float32)  # (8, 128, ncolh)
    full = np.empty((nd, D), np.float32)
    real = dom_core >= 0
    rc = dom_core[real]
    rh = dom_half[real]
    rcol = dom_hcol[real]
    rows = rh[:, None] * D + np.arange(D)[None, :]
    full[real] = outs[rc[:, None], rows, rcol[:, None]]
    if n0:
        full[~real] = b
    return full


# revision 20
# speedup vs baseline: 1.0759x; 1.0759x over previous
"""DOM pooling (segment mean+max over pulses, then linear projection) on 8 trn2 cores.

Strategy (v2, fp16 feature-major):
  Host: bucket DOMs by exact pulse count k ("classes"); deal DOMs of each
  class round-robin across the 8 cores so every core has identical structure
  (per-class m = ceil(n_k/8) doms, zero-padded). Windows of 128 doms per
  class, organized as two halves of <=64 doms. Slot buffers are fp16,
  feature-major: partition p = half*64 + embed, free = (window, slot, dom).
  Partial windows use D_w = ceil(rem/2) dom columns per half. No ragged
  sharing, no pad corrections (padding is all-zeros -> dummy doms only).

  Device (one NEFF, SPMD on 8 cores), per class window-group:
    - one contiguous fp16 DMA load per group (up to 32KB/partition runs)
    - segment SUM fused into the projection: k accumulating PE matmuls with
      block-diagonal fp16 weights blkdiag(W_sum/k) -> PSUM holds the
      projected mean contribution (scaling folded into weights)
    - segment MAX as a pairwise fp16 tensor_tensor tree on DVE (2x mode),
      then one more accumulating matmul with blkdiag(W_max) closes PSUM
    - ACT adds bias during PSUM->SBUF copy (fp16 out); per-group DMA store
  Partial windows compute both sum and max trees on DVE (tiny) + 2 matmuls.

  Host: scatter per-core outputs [128=(half,e), cols] back to (num_doms, 64).
"""
import sys

import numpy as np

for _p in ("/opt/trn_rl_repo",):
    if _p not in sys.path:
        sys.path.append(_p)

from concourse import bacc
import concourse.mybir as mybir
import concourse.tile as tile
from concourse.bass_utils import run_bass_kernel_spmd

NCORES = 8
D = 64
FP32 = mybir.dt.float32
FP16 = mybir.dt.float16

last_exec_ns = None  # set when KERNEL_TRACE=1


def _plan(counts):
    """Shared class/window structure (derived from global counts).

    Returns list of class dicts and totals. Per class k:
      m: doms per core (ceil), fw: full 128-dom windows, rem: leftover doms,
      dw: per-half columns of the partial window (ceil(rem/2)),
      base_f/base_p: slot-buffer element offsets (full / partial region),
      hcol0: first output column (per-half column space).
    """
    kmax = int(counts.max()) if counts.size else 0
    n_k = np.bincount(counts, minlength=kmax + 1)
    classes = []
    base = 0
    hcol = 0
    for k in range(1, kmax + 1):
        if n_k[k] == 0:
            continue
        m = -(-int(n_k[k]) // NCORES)
        fw = m // 128
        rem = m % 128
        dw = -(-rem // 2)
        c = dict(k=k, n=int(n_k[k]), m=m, fw=fw, rem=rem, dw=dw,
                 base_f=base, hcol0=hcol)
        base += 128 * fw * k * D
        hcol += fw * D
        c["base_p"] = base
        if rem:
            base += 128 * k * dw
            hcol += dw
        classes.append(c)
    return classes, base, hcol


def _emit_tree(eng, nc, v, k, gwa, dcols, out4, op, workp, tagp):
    """Pairwise reduction tree over the slot axis.

    v: 4D view (p, w=gwa, s=k, d=dcols); out4: (p, w, 1, d) destination view.
    Emits ceil(log2 k) fp16 tensor_tensor levels (+copies for odd carries).
    Caller must handle k == 1 (no op needed).
    """
    assert k >= 2
    cur = v
    s = k
    lvl = 0
    while s > 1:
        b = s // 2
        odd = s & 1
        tgt = b + odd
        if b == 1 and odd == 0:
            eng.tensor_tensor(out=out4[:, :, 0:1, :], in0=cur[:, :, 0:1, :],
                              in1=cur[:, :, 1:2, :], op=op)
            return
        wt = workp.tile([128, gwa * tgt * dcols], FP16, tag=f"{tagp}{lvl % 2}")
        dst = wt[:].rearrange("p (w s d) -> p w s d", w=gwa, s=tgt)
        eng.tensor_tensor(out=dst[:, :, 0:b, :], in0=cur[:, :, 0:b, :],
                          in1=cur[:, :, b : 2 * b, :], op=op)
        if odd:
            eng.tensor_copy(dst[:, :, b : b + 1, :], cur[:, :, 2 * b : 2 * b + 1, :])
        cur = dst
        s = tgt
        lvl += 1


def _build_nc(classes, s_elems, ncolh):
    ncls = len(classes)
    nc = bacc.Bacc(None)
    slots_t = nc.dram_tensor("slots", [s_elems], FP16, kind="ExternalInput")
    wts_t = nc.dram_tensor("wts", [(ncls + 1) * 128, 128], FP16, kind="ExternalInput")
    b_t = nc.dram_tensor("b", [128, 1], FP32, kind="ExternalInput")
    out_t = nc.dram_tensor("out", [128, ncolh], FP16, kind="ExternalOutput")

    with tile.TileContext(nc) as tc:
        with (
            tc.tile_pool(name="const", bufs=1) as constp,
            tc.tile_pool(name="inp", bufs=4) as inp,
            tc.tile_pool(name="pin", bufs=4) as pinp,
            tc.tile_pool(name="work", bufs=1) as workp,
            tc.tile_pool(name="redg", bufs=2) as redp,
            tc.tile_pool(name="outp", bufs=2) as outp,
            tc.tile_pool(name="ps", bufs=4, space="PSUM") as psp,
        ):
            wk_sb = constp.tile([128, (ncls + 1) * 128], FP16)
            nc.scalar.dma_start(
                wk_sb[:].rearrange("p (j m) -> p j m", m=128),
                wts_t[:, :].rearrange("(j p) m -> p j m", p=128),
            )
            b_sb = constp.tile([128, 1], FP32)
            nc.scalar.dma_start(b_sb[:], b_t[:])

            def w_ap(j):
                return wk_sb[:, j * 128 : (j + 1) * 128]

            wmax_j = ncls  # last weight block = blkdiag(W_max), unscaled

            def do_windows(jcls, k, in_t, gwa, dcols, base_col):
                """Reduce+project gwa windows of dcols half-columns each."""
                v = in_t[:].rearrange("p (w s d) -> p w s d", w=gwa, s=k)
                use_tree = k > 1
                if use_tree:
                    maxg = redp.tile([128, gwa * dcols], FP16, tag="maxg")
                    mg4 = maxg[:].rearrange("p (w s d) -> p w s d", w=gwa, s=1)
                    mg3 = mg4[:, :, 0, :]
                    _emit_tree(nc.vector, nc, v, k, gwa, dcols,
                               mg4, mybir.AluOpType.max, workp, "m")
                out_sb = outp.tile([128, gwa * dcols], FP16, tag="out")
                for w0 in range(0, gwa, 8):
                    sgw = min(8, gwa - w0)
                    N = sgw * dcols
                    ps = psp.tile([128, N], FP32, space="PSUM", tag="ps")
                    for s in range(k):
                        nc.tensor.matmul(
                            ps[:, :N], lhsT=w_ap(jcls),
                            rhs=v[:, w0 : w0 + sgw, s, :],
                            start=(s == 0), stop=False,
                        )
                    if use_tree:
                        rhs_max = mg3[:, w0 : w0 + sgw, :]
                    else:
                        rhs_max = v[:, w0 : w0 + sgw, 0, :]
                    nc.tensor.matmul(
                        ps[:, :N], lhsT=w_ap(wmax_j), rhs=rhs_max,
                        start=False, stop=True,
                    )
                    nc.scalar.activation(
                        out_sb[:, w0 * dcols : w0 * dcols + N], ps[:, :N],
                        mybir.ActivationFunctionType.Identity, bias=b_sb[:, :1],
                    )
                nc.gpsimd.dma_start(
                    out_t[:, base_col : base_col + gwa * dcols],
                    out_sb[:, : gwa * dcols],
                )

            def emit_partial(jcls):
                c = classes[jcls]
                k, fw, dw = c["k"], c["fw"], c["dw"]
                F0 = k * dw
                in_t = pinp.tile([128, F0], FP16, tag="pin")
                nc.sync.dma_start(
                    in_t[:],
                    slots_t[c["base_p"] : c["base_p"] + 128 * F0]
                    .rearrange("(p f) -> p f", p=128),
                )
                do_windows(jcls, k, in_t, 1, dw, c["hcol0"] + fw * D)

            # full-window groups (largest classes first), with the small
            # partial windows interleaved so their latency-bound chains hide
            # under the DMA-dense phase
            order = sorted(range(len(classes)),
                           key=lambda jj: -classes[jj]["fw"] * classes[jj]["k"])
            # interleave partials deepest-chain-first (descending k) so the
            # long tree+matmul chains hide under the DMA-dense phase and the
            # program tail is left with shallow work only
            head_p = sorted(
                (jj for jj, c in enumerate(classes) if c["rem"]),
                key=lambda jj: -classes[jj]["k"],
            )
            pi = 0
            for jcls in order:
                c = classes[jcls]
                k, fw = c["k"], c["fw"]
                if not fw:
                    continue
                gw = max(1, 192 // k)
                F_cls = fw * k * D
                full2d = slots_t[c["base_f"] : c["base_f"] + 128 * F_cls] \
                    .rearrange("(p f) -> p f", p=128)
                for g0 in range(0, fw, gw):
                    gwa = min(gw, fw - g0)
                    F0 = gwa * k * D
                    in_t = inp.tile([128, F0], FP16, tag="in")
                    nc.sync.dma_start(
                        in_t[:], full2d[:, g0 * k * D : g0 * k * D + F0]
                    )
                    do_windows(jcls, k, in_t, gwa, D, c["hcol0"] + g0 * D)
                    if pi < len(head_p):
                        emit_partial(head_p[pi])
                        pi += 1
            while pi < len(head_p):
                emit_partial(head_p[pi])
                pi += 1
    nc.finalize()
    return nc


def kernel(pulse_embeddings, pulse_to_dom_idx, num_doms, proj_w, proj_b):
    global last_exec_ns
    import os

    E = np.asarray(pulse_embeddings, dtype=np.float32)
    E16 = E.astype(np.float16)
    idx = np.asarray(pulse_to_dom_idx).astype(np.int64)
    nd = int(num_doms)
    W = np.asarray(proj_w, dtype=np.float32)   # (D, 2D)
    b = np.asarray(proj_b, dtype=np.float32)   # (D,)

    counts = np.bincount(idx, minlength=nd)
    classes, s_elems, ncolh = _plan(counts)
    ncls = len(classes)

    dom_order = np.argsort(counts, kind="stable")
    n0 = int((counts == 0).sum())
    perm = np.argsort(idx, kind="stable")
    pstart = np.zeros(nd + 1, np.int64)
    pstart[1:] = np.cumsum(counts)

    # per-dom output routing (core, half, halfcol) for real doms
    dom_core = np.full(nd, -1, np.int32)
    dom_half = np.zeros(nd, np.int32)
    dom_hcol = np.zeros(nd, np.int32)

    bufs = [np.zeros(s_elems, np.float16) for _ in range(NCORES)]
    off = n0
    for c in classes:
        k, n, m, fw, rem, dw = c["k"], c["n"], c["m"], c["fw"], c["rem"], c["dw"]
        doms_k = dom_order[off : off + n]
        off += n
        # routing: class-list index i -> core i%8, position p=i//8
        i_arr = np.arange(n, dtype=np.int64)
        p_arr = i_arr // NCORES
        dom_core[doms_k] = (i_arr % NCORES).astype(np.int32)
        isfull = p_arr < fw * 128
        q = np.where(isfull, p_arr % 128, p_arr - fw * 128)
        halfsz = np.where(isfull, 64, dw)
        dom_half[doms_k] = (q // halfsz).astype(np.int32)
        dcol = q % halfsz
        dom_hcol[doms_k] = np.where(
            isfull, c["hcol0"] + (p_arr // 128) * D + dcol,
            c["hcol0"] + fw * D + dcol,
        ).astype(np.int32)

        for cc in range(NCORES):
            doms_c = doms_k[cc::NCORES]
            nreal = len(doms_c)
            rows = pstart[doms_c][:, None] + np.arange(k)[None, :]
            X = E16[perm[rows]]  # (nreal, k, 64)
            if nreal < m:
                X = np.concatenate(
                    [X, np.zeros((m - nreal, k, D), np.float16)], axis=0
                )
            if fw:
                Xf = X[: fw * 128].reshape(fw, 2, 64, k, D)  # w h d s e
                arr = Xf.transpose(1, 4, 0, 3, 2)            # h e w s d
                bufs[cc][c["base_f"] : c["base_f"] + 128 * fw * k * D] = \
                    np.ascontiguousarray(arr).reshape(-1)
            if rem:
                Xr = X[fw * 128 :]  # (rem, k, D)
                if rem < 2 * dw:
                    Xr = np.concatenate(
                        [Xr, np.zeros((2 * dw - rem, k, D), np.float16)], axis=0
                    )
                arr = Xr.reshape(2, dw, k, D).transpose(0, 3, 2, 1)  # h e s d
                bufs[cc][c["base_p"] : c["base_p"] + 128 * k * dw] = \
                    np.ascontiguousarray(arr).reshape(-1)

    # ---- weights: per-class blkdiag(W_sum/k), plus blkdiag(W_max) --------
    Wsum = W[:, :D]   # (out_e, feat_e)
    Wmax = W[:, D:]
    wts = np.zeros(((ncls + 1) * 128, 128), np.float16)
    for j, c in enumerate(classes):
        blk = (Wsum.T / np.float32(c["k"])).astype(np.float16)  # (feat, out)
        wts[j * 128 : j * 128 + 64, 0:64] = blk
        wts[j * 128 + 64 : (j + 1) * 128, 64:128] = blk
    blk = Wmax.T.astype(np.float16)
    wts[ncls * 128 : ncls * 128 + 64, 0:64] = blk
    wts[ncls * 128 + 64 :, 64:128] = blk
    b_col = np.tile(b, 2).reshape(128, 1).astype(np.float32)

    # ---- device ----------------------------------------------------------
    nc = _build_nc(classes, s_elems, ncolh)
    in_maps = [{"slots": bufs[cc], "wts": wts, "b": b_col} for cc in range(NCORES)]
    trace = os.environ.get("KERNEL_TRACE", "0") == "1"
    kw_ = {}
    if trace:
        import tempfile
        kw_ = dict(trace=True, tmpdir=tempfile.mkdtemp(prefix="kernel_trace_"))
    res = run_bass_kernel_spmd(nc, in_maps, core_ids=list(range(NCORES)), **kw_)
    last_exec_ns = res.exec_time_ns

    # ---- host-side unpermute --------------------------------------------
    outs = np.stack([res.results[cc]["out"] for cc in range(NCORES)]) \
        .astype(np.float32)  # (8, 128, ncolh)
    full = np.empty((nd, D), np.float32)
    real = dom_core >= 0
    rc = dom_core[real]
    rh = dom_half[real]
    rcol = dom_hcol[real]
    rows = rh[:, None] * D + np.arange(D)[None, :]
    full[real] = outs[rc[:, None], rows, rcol[:, None]]
    if n0:
        full[~real] = b
    return full
